# revision 31
# baseline (speedup 1.0000x reference)
"""GCNConv (SpMM + dense projection) on 8 Trainium2 NeuronCores.

out[i] = deg[i] * sum_{e in CSR row i} deg[col_e] * (X @ W)[col_e]
       = deg[i] * ( (sum_{e} deg[col_e] * X[col_e]) @ W )        (linearity)

Strategy (SPMD single program, per-core data):
  - Rows (outputs) are sharded: core c owns rows [c*12500, (c+1)*12500),
    padded to 12544 = 98 windows of 128 rows.
  - X (f32 or bf16) is replicated in each core's DRAM; edge gathers are done
    with GPSIMD dma_gather (int16 indices => X is split into 4 chunks of
    25000 rows, edges bucketed by chunk on host).
  - Edges are host-bucketed into a fixed, capacity-padded schedule of
    (super-batch of SBW windows) x (4 chunks) x (window) sub-buckets,
    capacities = max over cores, so all 8 cores run the same program and all
    data-dependence lives in tensor contents (indices / rowids / coefs).
  - X is pre-scaled by deg on the host, so per 128-edge group DVE builds the
    pure one-hot S[e, r] = (iota == rowid_e); PE accumulates
    A^T[f, r] += G^T @ S into PSUM.  Per window: PE computes (A^T)^T @ W,
    DVE scales rows by deg[row] during PSUM evacuation.  Rows are LPT-
    balanced across (core, window) bins so every (window, chunk) bucket is
    exactly 4 groups (near-zero padding); the host unshard permutation
    restores row order.
"""

import os
import sys

sys.path.insert(0, "/opt/trn_rl_repo")

import numpy as np

N = 100000
E = 1600000
D = 128
NCORES = 8
RPC = 12500            # rows per core
NWIN = 98              # ceil(12500/128) windows per core
ROWS_PAD = NWIN * 128  # 12544
NCHUNK = 4
CHUNK = 25000

GATHER_DT = os.environ.get("GCN_GATHER_DT", "bf16")  # "f32" | "bf16"
# Max indices per dma_gather: the SWDGE descriptor ring holds
# dynamic_dma_scratch_size/16 descriptors and single_packet=True needs the
# whole gather resident. single_packet=False lifts the cap but measured 4.7x
# slower end-to-end (per-packet SDMA overhead).
SCRATCH = int(os.environ.get("GCN_SCRATCH", "16384"))
GMAX = int(os.environ.get("GCN_GMAX", "1024"))
SBW = int(os.environ.get("GCN_SBW", "8" if GATHER_DT == "bf16" else "4"))
NSB = (NWIN + SBW - 1) // SBW
NQUEUES = int(os.environ.get("GCN_QUEUES", "4"))  # SWDGE queues for gathers
SORT_COLS = bool(int(os.environ.get("GCN_SORT", "1")))  # sort buckets by col
BALANCE = bool(int(os.environ.get("GCN_BALANCE", "1")))  # LPT row->bin balance
NREP = int(os.environ.get("GCN_NREP", "1"))  # repeat body (timing calibration)
SKIP_COMPUTE = bool(int(os.environ.get("GCN_SKIP_COMPUTE", "0")))  # diagnostics
SKIP_GATHER = bool(int(os.environ.get("GCN_SKIP_GATHER", "0")))  # diagnostics

_cache = {}


def _balance_rows(row_id, ci):
    """LPT-assign rows to (core, window) bins so every (bin, chunk) edge
    count stays <= 512 (= 4 full 128-edge matmul groups, near-zero padding).
    Legal because the host controls the output unshard permutation."""
    chunk_of = ci // CHUNK
    d = (
        np.bincount(row_id * NCHUNK + chunk_of, minlength=N * NCHUNK)
        .reshape(N, NCHUNK)
        .astype(np.float64)
    )
    tot = d.sum(1)
    NBINS = NCORES * NWIN
    CAP = 512.0
    L = np.zeros((NBINS, NCHUNK), np.float64)
    L2 = np.zeros(NBINS, np.float64)
    S = np.zeros(NBINS, np.int64)
    binof = np.empty(N, np.int32)
    full = np.zeros(NBINS, np.float64)
    order = np.argsort(-tot, kind="stable")
    for i in order:
        di = d[i]
        cost = L2 + 2.0 * (L @ di)
        over = np.maximum(L + di - CAP, 0.0).sum(1)
        b = int(np.argmin(cost + 1e6 * over + full))
        binof[i] = b
        L[b] += di
        L2[b] = float((L[b] ** 2).sum())
        S[b] += 1
        if S[b] >= 128:
            full[b] = 1e12
    # relabel windows within each core by descending load so window w has
    # similar load on every core (shrinks the max-over-cores capacity)
    core_of = np.empty(N, np.int32)
    local_of = np.empty(N, np.int32)
    for c in range(NCORES):
        bins = np.arange(c * NWIN, (c + 1) * NWIN)
        win_rank = np.empty(NWIN, np.int64)
        win_rank[np.argsort(-L[bins].sum(1), kind="stable")] = np.arange(NWIN)
        rows_c = np.nonzero((binof >= c * NWIN) & (binof < (c + 1) * NWIN))[0]
        w_new = win_rank[binof[rows_c] - c * NWIN]
        o = np.argsort(w_new, kind="stable")
        rows_o = rows_c[o]
        w_o = w_new[o]
        slot = np.arange(len(rows_o)) - np.searchsorted(w_o, w_o)
        core_of[rows_o] = c
        local_of[rows_o] = (w_o * 128 + slot).astype(np.int32)
    return core_of, local_of


def _build_schedule(degrees, row_pointers, column_index):
    """Host-side graph partitioning: per-core capacity-padded edge buckets."""
    rp = np.asarray(row_pointers, dtype=np.int64)
    ci = np.asarray(column_index, dtype=np.int64)
    deg = np.asarray(degrees, dtype=np.float32)

    row_id = np.searchsorted(rp, np.arange(E, dtype=np.int64), side="right") - 1

    if BALANCE:
        core_of, local_of = _balance_rows(row_id, ci)
    else:
        rows = np.arange(N, dtype=np.int64)
        core_of = (rows // RPC).astype(np.int32)
        local_of = (rows - (rows // RPC) * RPC).astype(np.int32)

    edge_core = core_of[row_id]
    edge_lr = local_of[row_id]

    NKEY = NSB * NCHUNK * SBW
    counts = np.zeros((NCORES, NKEY), dtype=np.int64)
    percore = []
    for c in range(NCORES):
        mask = edge_core == c
        lr = edge_lr[mask].astype(np.int32)
        cols = ci[mask].astype(np.int32)
        win = lr >> 7
        sb = win // SBW
        wl = win - sb * SBW
        chunk = cols // CHUNK
        key = (sb * NCHUNK + chunk) * SBW + wl
        if SORT_COLS:
            order = np.lexsort((cols, key))
        else:
            order = np.argsort(key, kind="stable")
        key_s = key[order]
        counts[c] = np.bincount(key_s, minlength=NKEY)
        percore.append((lr[order], cols[order], key_s))

    cap = counts.max(axis=0)
    cap = ((cap + 127) // 128) * 128
    cap3 = cap.reshape(NSB, NCHUNK, SBW)
    # ensure every (sb, wl) window has at least one group (to zero its PSUM)
    for sb in range(NSB):
        nw = min(SBW, NWIN - sb * SBW)
        for wl in range(nw):
            if cap3[sb, :, wl].sum() == 0:
                cap3[sb, 0, wl] = 128
    cap = cap3.reshape(-1)

    slot_off = np.zeros(NKEY + 1, dtype=np.int64)
    np.cumsum(cap, out=slot_off[1:])
    totcap = int(slot_off[-1])
    ngroups_tot = totcap // 128

    idx16 = np.zeros((NCORES, 128, totcap // 16), dtype=np.int16)
    rowid = np.zeros((NCORES, 128, ngroups_tot), dtype=np.float32)
    degw = np.zeros((NCORES, 128, NWIN), dtype=np.float32)

    for c in range(NCORES):
        lr, cols, key_s = percore[c]
        bstart = np.zeros(NKEY, dtype=np.int64)
        bstart[1:] = np.cumsum(counts[c])[:-1]
        pos = np.arange(len(key_s)) - bstart[key_s]
        dest = slot_off[key_s] + pos

        idx_flat = np.zeros(totcap, dtype=np.int16)
        # pad slots get rowid=-1: is_equal(iota, -1) matches nothing, so the
        # gathered garbage in pad slots contributes zero (X is pre-scaled by
        # deg on the host, so no per-edge coef is needed for real slots)
        rid_flat = np.full(totcap, -1.0, dtype=np.float32)
        chunk_of = (key_s // SBW) % NCHUNK
        sbwl = key_s // (NCHUNK * SBW) * SBW + key_s % SBW  # global window
        idx_flat[dest] = (cols - chunk_of * CHUNK).astype(np.int16)
        rid_flat[dest] = (lr - sbwl * 128).astype(np.float32)

        idx16[c] = np.tile(idx_flat.reshape(-1, 16).T, (8, 1))
        rowid[c] = rid_flat.reshape(-1, 128).T

        dpad = np.zeros(ROWS_PAD, dtype=np.float32)
        rows_c = np.nonzero(core_of == c)[0]
        dpad[local_of[rows_c]] = deg[rows_c]
        degw[c] = dpad.reshape(NWIN, 128).T

    return (
        cap.reshape(NSB, NCHUNK, SBW), slot_off, idx16, rowid, degw,
        core_of, local_of,
    )


def _build_bass(cap3, slot_off):
    import concourse.bacc as bacc
    import concourse.mybir as mybir
    import concourse.tile as tile

    sdt = mybir.dt.float32 if GATHER_DT == "f32" else mybir.dt.bfloat16

    totcap = int(slot_off[-1])
    gtot = totcap // 128

    nc = bacc.Bacc(
        "TRN2",
        target_bir_lowering=False,
        num_swdge_queues=NQUEUES,
        dynamic_dma_scratch_size=SCRATCH,
    )
    x_d = nc.dram_tensor("x", [N, D], sdt, kind="ExternalInput")
    w_d = nc.dram_tensor("w", [D, D], mybir.dt.float32, kind="ExternalInput")
    iota_d = nc.dram_tensor("iota", [128, 128], sdt, kind="ExternalInput")
    idx_d = nc.dram_tensor("idx", [128, totcap // 16], mybir.dt.int16, kind="ExternalInput")
    rowid_d = nc.dram_tensor("rowid", [128, gtot], sdt, kind="ExternalInput")
    degw_d = nc.dram_tensor("degw", [128, NWIN], mybir.dt.float32, kind="ExternalInput")
    out_d = nc.dram_tensor("out", [ROWS_PAD, D], mybir.dt.float32, kind="ExternalOutput")

    # max capacity per chunk tag across super-batches (for pool slot sizing)
    with tile.TileContext(nc) as tc:
        with tc.tile_pool(name="const", bufs=1) as cpool, \
             tc.tile_pool(name="gp", bufs=2) as gpool, \
             tc.tile_pool(name="sp", bufs=12) as spool, \
             tc.tile_pool(name="ep", bufs=2) as epool, \
             tc.tile_pool(name="at_ps", bufs=2, space="PSUM") as atpool, \
             tc.tile_pool(name="o_ps", bufs=2, space="PSUM") as opool:

            w_sb = cpool.tile([D, D], mybir.dt.float32, tag="w")
            nc.sync.dma_start(w_sb[:, :], w_d[:, :])
            iota_sb = cpool.tile([128, 128], sdt, tag="iota")
            nc.sync.dma_start(iota_sb[:, :], iota_d[:, :])
            degw_sb = cpool.tile([128, NWIN], mybir.dt.float32, tag="degw")
            nc.sync.dma_start(degw_sb[:, :], degw_d[:, :])
            idx_sb = cpool.tile([128, totcap // 16], mybir.dt.int16, tag="idx")
            nc.sync.dma_start(idx_sb[:, :], idx_d[:, :])
            rowid_sb = cpool.tile([128, gtot], sdt, tag="rowid")
            nc.sync.dma_start(rowid_sb[:, :], rowid_d[:, :])

            gq = 0  # round-robin gather queue counter
            for sb in [s for _ in range(NREP) for s in range(NSB)]:
                nw = min(SBW, NWIN - sb * SBW)
                # per-chunk gathers for this super-batch
                g_tiles = {}
                base_off = {}
                for ch in range(NCHUNK):
                    csum = int(cap3[sb, ch, :].sum())
                    if csum == 0:
                        continue
                    off = int(slot_off[(sb * NCHUNK + ch) * SBW])
                    base_off[ch] = off
                    gt = gpool.tile([128, csum // 128, D], sdt, tag=f"g{ch}")
                    if not SKIP_GATHER:
                        for j0 in range(0, csum, GMAX):
                            n_j = min(GMAX, csum - j0)
                            nc.gpsimd.dma_gather(
                                gt[:, j0 // 128 : (j0 + n_j) // 128, :],
                                x_d[ch * CHUNK : (ch + 1) * CHUNK, :],
                                idx_sb[:, (off + j0) // 16 : (off + j0 + n_j) // 16],
                                n_j, n_j, D,
                                queue_num=gq % NQUEUES,
                            )
                            gq += 1
                    g_tiles[ch] = gt

                if SKIP_COMPUTE:
                    continue
                at_ps = atpool.tile([128, SBW * 128], mybir.dt.float32, tag="at")
                for wl in range(nw):
                    buckets = []
                    for ch in range(NCHUNK):
                        ng = int(cap3[sb, ch, wl]) // 128
                        if ng == 0:
                            continue
                        gcol0 = int(slot_off[(sb * NCHUNK + ch) * SBW + wl]) // 128
                        gslot0 = gcol0 - base_off.get(ch, 0) // 128
                        buckets.append((ch, gslot0, gcol0, ng))
                    total_g = sum(b[3] for b in buckets)
                    gi = 0
                    for ch, gslot0, gcol0, ng in buckets:
                        # one blockwise one-hot build per <=4-group bucket:
                        # S[p, j, r] = (iota[p, r] == rowid[p, gcol0+b0+j])
                        for b0 in range(0, ng, 4):
                            nb = min(4, ng - b0)
                            s_t = spool.tile([128, 4, 128], sdt, tag="s")
                            nc.vector.tensor_tensor(
                                s_t[:, 0:nb, :],
                                iota_sb[:, :].unsqueeze(1)
                                .to_broadcast([128, nb, 128]),
                                rowid_sb[:, gcol0 + b0 : gcol0 + b0 + nb]
                                .unsqueeze(2).to_broadcast([128, nb, 128]),
                                mybir.AluOpType.is_equal,
                            )
                            for j in range(nb):
                                nc.tensor.matmul(
                                    at_ps[:, wl * 128 : (wl + 1) * 128],
                                    g_tiles[ch][:, gslot0 + b0 + j, :],
                                    s_t[:, j, :],
                                    start=(gi == 0), stop=(gi == total_g - 1),
                                )
                                gi += 1

                at_sb = epool.tile([128, SBW * 128], mybir.dt.float32, tag="atsb")
                nc.vector.tensor_copy(at_sb[:, : nw * 128], at_ps[:, : nw * 128])
                stage = epool.tile([128, SBW * 128], mybir.dt.float32, tag="stage")
                for wl in range(nw):
                    o_ps = opool.tile([128, 128], mybir.dt.float32, tag="o")
                    nc.tensor.matmul(
                        o_ps[:, :], at_sb[:, wl * 128 : (wl + 1) * 128],
                        w_sb[:, :], start=True, stop=True,
                    )
                    w_glob = sb * SBW + wl
                    nc.vector.tensor_scalar(
                        stage[:, wl * 128 : (wl + 1) * 128], o_ps[:, :],
                        degw_sb[:, w_glob : w_glob + 1], None,
                        mybir.AluOpType.mult,
                    )
                for wl in range(nw):
                    w_glob = sb * SBW + wl
                    nc.sync.dma_start(
                        out_d[w_glob * 128 : (w_glob + 1) * 128, :],
                        stage[:, wl * 128 : (wl + 1) * 128],
                    )

            if SKIP_COMPUTE:
                nc.sync.dma_start(out_d[0:128, :], w_sb[:, :])

    nc.compile()
    return nc


def build_in_maps(X, weights, degrees, idx16, rowid, degw):
    X = np.asarray(X)
    weights = np.asarray(weights, dtype=np.float32)
    degrees = np.asarray(degrees, dtype=np.float32)
    xs = X * degrees[:, None]  # pre-scale by source-side deg (host)
    if GATHER_DT == "f32":
        sdt_np = np.float32
    else:
        import ml_dtypes

        sdt_np = ml_dtypes.bfloat16
    xg = xs.astype(sdt_np)
    iota = np.tile(np.arange(128, dtype=sdt_np), (128, 1))
    return [
        {
            "x": xg, "w": weights, "iota": iota,
            "idx": idx16[c], "rowid": rowid[c].astype(sdt_np),
            "degw": degw[c],
        }
        for c in range(NCORES)
    ]


def kernel(X, weights, degrees, row_pointers, column_index):
    from concourse.bass_utils import run_bass_kernel_spmd

    try:  # persistent NEFF-embedding compile cache (keyed on full BIR)
        import jax

        jax.config.update("jax_compilation_cache_dir", "/tmp/jaxcache")
        jax.config.update("jax_persistent_cache_min_entry_size_bytes", -1)
        jax.config.update("jax_persistent_cache_min_compile_time_secs", 0)
    except Exception:
        pass

    X = np.asarray(X)
    weights = np.asarray(weights, dtype=np.float32)
    degrees = np.asarray(degrees, dtype=np.float32)

    cap3, slot_off, idx16, rowid, degw, core_of, local_of = _build_schedule(
        degrees, row_pointers, column_index
    )

    key = (
        GATHER_DT, SBW, NQUEUES, GMAX, SCRATCH, NREP,
        SKIP_COMPUTE, SKIP_GATHER, cap3.tobytes(),
    )
    if key not in _cache:
        _cache.clear()
        _cache[key] = _build_bass(cap3, slot_off)
    nc = _cache[key]

    in_maps = build_in_maps(X, weights, degrees, idx16, rowid, degw)

    trace = bool(int(os.environ.get("GCN_TRACE", "0")))
    last_err = None
    for attempt in range(3):
        try:
            res = run_bass_kernel_spmd(
                nc, in_maps, core_ids=list(range(NCORES)), trace=trace
            )
            # rare transient device corruption has been observed (NaNs in
            # otherwise-correct runs); detect and retry
            if all(np.isfinite(res.results[c]["out"]).all() for c in range(NCORES)):
                break
            last_err = RuntimeError("non-finite kernel output")
        except Exception as e:  # transient device-unrecoverable on cold start
            last_err = e
            import time as _time

            try:  # a dead PJRT session can't recover in-place; reset backends
                import jax

                jax.clear_caches()
                jax._src.api.clear_backends()
            except Exception:
                pass
            _time.sleep(10)
    else:
        raise last_err
    global last_results
    last_results = res

    out = np.empty((N, D), dtype=np.float32)
    for c in range(NCORES):
        rows_c = np.nonzero(core_of == c)[0]
        out[rows_c] = res.results[c]["out"][local_of[rows_c]]
    return out


last_results = None

